# revision 3
# baseline (speedup 1.0000x reference)
"""Trainium2 Bass kernel for nn_Attention — v4.

Backend is a serial per-core instruction interpreter: wall time ~= sum of
per-instruction (Python dispatch + numpy) costs; engines do not overlap.
Kernel is structured to minimize per-core instruction count:
  - f32/f32r operands everywhere (f32r for matmul operands; plain-f32
    matmuls crash at scale)
  - RMS norm via partition_all_reduce (1 instr)
  - V^T produced by a DRAM-roundtrip transpose (internal scratch tensor,
    not downloaded)
  - K/V/Q projections share one PSUM pool/buffer (fewer scope sems,
    hot accumulator)
  - softmax denominators via the ones-column trick; normalization via
    reciprocal + partition_broadcast (3 instrs per head-half)
  - attention loop at the ISA floor: 4 QK + 1 exp + 4 AV per
    (head, key-block), N=512 per matmul (PSUM bank cap); exp covers 3
    key-blocks ([128,3072], 6 banks) with AV accumulator [33,1024] (2 banks)

Sharding: core = (batch b, query-half t). Each core gets x[b] rolled so
its 2048 query tokens come first; keys/values cover all 4096 tokens
(order-invariant under softmax).
"""
import numpy as np

HEADS = 4
HD = 32
DIM = 256
N = 4096
NQ = 2048
EPS = 1e-12
N_CORES = 8

_cache = {}


def _build(reps: int = 1):
    import concourse.tile as tile
    from concourse import bacc, mybir, bass_isa
    from concourse.tile_rust import add_dep_helper

    F32 = mybir.dt.float32
    F32R = mybir.dt.float32r
    AF = mybir.ActivationFunctionType

    nc = bacc.Bacc("TRN2", target_bir_lowering=False, debug=False,
                   num_devices=N_CORES)

    x_in = nc.dram_tensor("x", [DIM, N], F32, kind="ExternalInput")
    wb_in = nc.dram_tensor("wb", [128, 1026], F32, kind="ExternalInput")
    out_dram = nc.dram_tensor("out", [DIM, NQ], F32, kind="ExternalOutput")
    scr_dram = nc.dram_tensor("scr", [N, 132], mybir.dt.float32r, kind="Internal")

    prev_vt_read = None
    scr_v_g = scr_dram.rearrange("t (h dd) -> h dd t", h=4)

    with tile.TileContext(nc) as tc:
      with tc.tile_pool(name="consts", bufs=1) as cpool:
        # rep-invariant setup: weights blob, eps, scratch ones row
        wb = cpool.tile([128, 1026], F32, tag="wb")
        nc.sync.dma_start(out=wb, in_=wb_in[:, :])
        wbr = cpool.tile([128, 1024], F32R, tag="wbr")
        nc.vector.tensor_copy(wbr, wb[:, 0:1024])
        wqt = wbr[:, 0:256].rearrange("p (cc m) -> p cc m", cc=2)
        wkt = wbr[:, 256:512].rearrange("p (cc m) -> p cc m", cc=2)
        wvt = wbr[:, 512:768].rearrange("p (cc m) -> p cc m", cc=2)
        wot = wbr[:, 768:1024]
        bo2 = wb[:, 1024:1026]
        eps_t = cpool.tile([128, 1], F32, tag="eps")
        nc.vector.memset(eps_t, EPS)
        with tc.tile_pool(name="onetmp", bufs=1) as otp:
            ones4f = otp.tile([4, N], F32, tag="ones4f")
            nc.vector.memset(ones4f, 1.0)
            ones4 = otp.tile([4, N], F32R, tag="ones4")
            nc.vector.tensor_copy(ones4, ones4f)
            ones_w = nc.sync.dma_start(out=scr_v_g[:, 32, :], in_=ones4)

        for rep in range(reps):
          with tc.tile_pool(name=f"mp{rep}", bufs=1) as mp:

              xn = mp.tile([128, 2, N], F32R, tag="xn")

              # ---- load x + RMS statistics -> xn = x * invrms ----
              with tc.tile_pool(name=f"ld{rep}", bufs=1) as ldp:
                  xf = ldp.tile([128, 2, N], F32, tag="xf")
                  nc.sync.dma_start(
                      out=xf, in_=x_in.rearrange("(cc p) m -> p cc m", p=128))
                  with tc.tile_pool(name=f"sq{rep}", bufs=1) as sqp:
                      xsq = sqp.tile([128, 2, N], F32, tag="xsq")
                      nc.vector.tensor_mul(xsq, xf, xf)
                      msq = ldp.tile([128, N], F32, tag="t16a", name="msq")
                      nc.vector.tensor_add(msq, xsq[:, 0, :], xsq[:, 1, :])
                  ssum = ldp.tile([128, N], F32, tag="t16b", name="ssum")
                  nc.gpsimd.partition_all_reduce(
                      ssum, msq, channels=128, reduce_op=bass_isa.ReduceOp.add)
                  rms = ldp.tile([128, N], F32, tag="t16a", name="rms")
                  nc.scalar.activation(rms, ssum, AF.Sqrt, scale=1.0 / DIM,
                                       bias=eps_t)
                  inv = ldp.tile([128, N], F32, tag="t16b", name="inv")
                  nc.vector.reciprocal(inv, rms)
                  for cc in range(2):
                      nc.vector.tensor_mul(xn[:, cc, :], xf[:, cc, :], inv)

              # ---- projections ----
              kr = mp.tile([128, N], F32R, tag="kr")
              qr = mp.tile([128, NQ], F32R, tag="qr")
              vt = mp.tile([128, 32, 132], F32R, tag="vt")

              vt_writes = []
              scr_v = scr_v_g
              with tc.tile_pool(name=f"vsp{rep}", bufs=1) as vsp, \
                   tc.tile_pool(name=f"psp{rep}", bufs=1, space="PSUM") as psp:
                  pps = psp.tile([128, N], F32, tag="big")
                  for nb_ in range(8):
                      sl = slice(512 * nb_, 512 * (nb_ + 1))
                      for cc in range(2):
                          nc.tensor.matmul(pps[:, sl], wkt[:, cc, :],
                                           xn[:, cc, sl],
                                           start=(cc == 0), stop=(cc == 1))
                  nc.vector.tensor_copy(kr, pps)
                  for nb_ in range(8):
                      sl = slice(512 * nb_, 512 * (nb_ + 1))
                      for cc in range(2):
                          nc.tensor.matmul(pps[:, sl], wvt[:, cc, :],
                                           xn[:, cc, sl],
                                           start=(cc == 0), stop=(cc == 1))
                  vs = vsp.tile([128, N], F32R, tag="vs")
                  nc.vector.tensor_copy(vs, pps)
                  for h in range(4):
                      w = nc.sync.dma_start(out=scr_v[h, 0:32, :],
                                            in_=vs[32 * h:32 * h + 32, :])
                      vt_writes.append(w)
                  for nb_ in range(4):
                      sl = slice(512 * nb_, 512 * (nb_ + 1))
                      for cc in range(2):
                          nc.tensor.matmul(pps[:, sl], wqt[:, cc, :],
                                           xn[:, cc, sl],
                                           start=(cc == 0), stop=(cc == 1))
                  nc.vector.tensor_copy(qr, pps[:, 0:NQ])

              # V^T readback (transposed via DRAM layout)
              r_i = nc.scalar.dma_start(
                  out=vt, in_=scr_dram.rearrange("(jb t) hd -> t jb hd", t=128))
              add_dep_helper(r_i.ins, ones_w.ins, sync=True,
                             reason="ones row write-before-read")
              for w in vt_writes:
                  add_dep_helper(r_i.ins, w.ins, sync=True,
                                 reason="vt transpose write-before-read")
                  if prev_vt_read is not None:
                      add_dep_helper(w.ins, prev_vt_read.ins, sync=True,
                                     reason="vt cross-rep read-before-write")
              prev_vt_read = r_i

              # ---- attention ----
              # Widened exp: one activation covers up to 3 key-blocks
              # ([128, 3072] PSUM, 6 banks) with the AV accumulator shrunk
              # to [33, 1024] (2 banks, per query-half) -> 88 acts not 128.
              on = mp.tile([128, NQ], F32R, tag="on")
              GROUPS = [list(range(3 * g, min(3 * g + 3, 32)))
                        for g in range((32 + 2) // 3)]
              with tc.tile_pool(name=f"att{rep}", bufs=1) as ap_, \
                   tc.tile_pool(name=f"psa{rep}", bufs=1, space="PSUM") as psa:
                  for h in range(4):
                    for qh in range(2):
                      qb = 1024 * qh
                      oh = psa.tile([33, 1024], F32, tag="oh",
                                    name=f"oh{rep}_{h}_{qh}")
                      for g, jbs in enumerate(GROUPS):
                          s4 = psa.tile([128, 3072], F32, tag="s4",
                                        name=f"s4_{rep}_{h}_{qh}_{g}")
                          for i, jb in enumerate(jbs):
                              for c2 in range(2):
                                  so = slice(1024 * i + 512 * c2,
                                             1024 * i + 512 * (c2 + 1))
                                  sq = slice(qb + 512 * c2,
                                             qb + 512 * (c2 + 1))
                                  nc.tensor.matmul(
                                      s4[:, so],
                                      kr[32 * h:32 * h + 32,
                                         128 * jb:128 * (jb + 1)],
                                      qr[32 * h:32 * h + 32, sq],
                                      start=True, stop=True,
                                      tile_position=(32 * h, 0))
                          width = 1024 * len(jbs)
                          e4 = ap_.tile([128, 3072], F32R, tag="e4",
                                        name=f"e4_{rep}_{h}_{qh}_{g}")
                          nc.scalar.activation(e4[:, 0:width], s4[:, 0:width],
                                               AF.Exp)
                          for i, jb in enumerate(jbs):
                              for c2 in range(2):
                                  se = slice(1024 * i + 512 * c2,
                                             1024 * i + 512 * (c2 + 1))
                                  so2 = slice(512 * c2, 512 * (c2 + 1))
                                  nc.tensor.matmul(
                                      oh[:, so2],
                                      vt[:, jb, 33 * h:33 * h + 33],
                                      e4[:, se],
                                      start=(g == 0 and i == 0),
                                      stop=(g == len(GROUPS) - 1
                                            and i == len(jbs) - 1))
                      rcp = ap_.tile([1, 1024], F32, tag="rcp",
                                     name=f"rcp{rep}_{h}_{qh}")
                      nc.vector.reciprocal(rcp, oh[32:33, :])
                      nbt = ap_.tile([32, 1024], F32, tag="nbt",
                                     name=f"nbt{rep}_{h}_{qh}")
                      nc.gpsimd.partition_broadcast(nbt, rcp)
                      nc.vector.tensor_mul(
                          on[32 * h:32 * h + 32, qb:qb + 1024],
                          oh[0:32, :], nbt)

              # ---- output projection + bias ----
              osb = mp.tile([128, 2, NQ], F32, tag="osb")
              with tc.tile_pool(name=f"pso{rep}", bufs=1, space="PSUM") as pso:
                  for oc in range(2):
                      pp = pso.tile([128, NQ], F32, tag="pp",
                                    name=f"pp{rep}_{oc}")
                      for c4 in range(4):
                          sl = slice(512 * c4, 512 * (c4 + 1))
                          nc.tensor.matmul(pp[:, sl],
                                           wot[:, 128 * oc:128 * (oc + 1)],
                                           on[:, sl], start=True, stop=True)
                      nc.vector.tensor_scalar_add(osb[:, oc, :], pp,
                                                  bo2[:, oc:oc + 1])
              nc.sync.dma_start(
                  out=out_dram.rearrange("(oc p) m -> p oc m", p=128), in_=osb)

    nc.compile()
    return nc


def _get_nc(reps: int = 1):
    if reps not in _cache:
        _cache[reps] = _build(reps)
    return _cache[reps]


def _prep_inputs(x, g, w_qkv, w_out, b_out):
    x = np.asarray(x, np.float32)
    g = np.asarray(g, np.float32)
    w_qkv = np.asarray(w_qkv, np.float32)
    w_out = np.asarray(w_out, np.float32)
    b_out = np.asarray(b_out, np.float32)

    wqg = (w_qkv[0:128] * g[None, :]) * (HD ** -0.5)   # [128, 256]
    wkg = w_qkv[128:256] * g[None, :]
    wvg = w_qkv[256:384] * g[None, :]

    wb = np.empty((128, 1026), np.float32)
    # wqt[p, cc, m] = wqg[m, 128*cc + p]
    for off, wmat in ((0, wqg), (256, wkg), (512, wvg)):
        t = wmat.reshape(128, 2, 128).transpose(2, 1, 0)  # [p, cc, m]
        wb[:, off:off + 256] = t.reshape(128, 256)
    wb[:, 768:1024] = w_out.T                      # wot[p, m] = w_out[m, p]
    wb[:, 1024:1026] = b_out.reshape(2, 128).T     # bo2[p, oc] = b_out[128*oc+p]

    b, c, hh, ww = x.shape
    xf = x.reshape(b, c, hh * ww)
    in_maps = []
    for core in range(N_CORES):
        beta, tau = core // 2, core % 2
        xr = np.concatenate([xf[beta][:, NQ * tau:], xf[beta][:, :NQ * tau]],
                            axis=1)
        in_maps.append({
            "x": np.ascontiguousarray(xr),
            "wb": wb,
        })
    return in_maps


def _run(in_maps, reps: int = 1):
    from concourse.bass_utils import run_bass_kernel_spmd
    nc = _get_nc(reps)
    return run_bass_kernel_spmd(nc, in_maps, list(range(N_CORES))).results


def kernel(x, g, w_qkv, w_out, b_out):
    x = np.asarray(x, np.float32)
    b, c, hh, ww = x.shape
    in_maps = _prep_inputs(x, g, w_qkv, w_out, b_out)
    results = _run(in_maps, reps=1)
    out = np.empty((b, DIM, hh * ww), np.float32)
    for core in range(N_CORES):
        beta, tau = core // 2, core % 2
        out[beta][:, NQ * tau:NQ * (tau + 1)] = results[core]["out"]
    return out.reshape(b, DIM, hh, ww)

